# revision 15
# baseline (speedup 1.0000x reference)
"""GPT-OSS sliding-window attention (B=2, S=2048, M=4096, 32 q-heads / 8 kv-heads,
window=1024, attention sinks) on 8 trn2 NeuronCores.

Sharding: core = (batch b, head-group g) with b = core//4, g = core%4.
Each core computes 8 q-heads (2 kv-heads) over the full sequence for its batch,
projects through its Wo slice, and the 4 cores of a batch ReduceScatter the
partial [2048, 4096] outputs over the sequence dim -> each core owns disjoint
output rows.  Host-side unshard is a pure gather.

All matmuls run in fp16 (moving/stationary operands) with fp32 PSUM
accumulation; softmax (exp / denominator / reciprocal) in fp32.  Scores are
computed transposed ([keys, q]) so probabilities feed the PV and the
denominator (all-ones lhsT) matmuls directly, with no transposes anywhere.
The causal + sliding-window structure of the mask is exploited: only key-tiles
intersecting the window are computed, and only the 8 boundary-tile patterns
(4 causal-edge + 4 window-edge) are masked, via constant 0/1 fp16 tiles.
"""

import numpy as np

B, S, M = 2, 2048, 4096
NQ, NKV, HD = 32, 8, 128
WINDOW = 1024
MAX_WAVELENGTH = 10000.0
INV_NORM = 1.0 / np.sqrt(HD)
SHIFT = 6.0          # softmax logit shift: exp(s/sqrt(d) - SHIFT), folded into denom+sinks
NC_CORES = 8
HG = 8               # q heads per core
KVG = 2              # kv heads per core
C = 4                # q chunks per sequence
TQ = S // C          # 512 tokens per chunk
NMT = M // 128       # 32 contraction tiles for projections
NKT = S // 128       # 16 key tiles
GROUPS = [[0, 1, 2, 3], [4, 5, 6, 7]]

_built = {}


def _key_tiles(c):
    """Key-tile indices intersecting the causal+sliding window of chunk c."""
    return list(range(max(0, 4 * c - 8), 4 * c + 4))


def _build():
    import concourse.bass as bass
    import concourse.tile as tile
    from concourse import bacc, mybir

    dt = mybir.dt
    f32, f16 = dt.float32, dt.float16
    Exp = mybir.ActivationFunctionType.Exp

    nc = bacc.Bacc("TRN2", target_bir_lowering=False, debug=False,
                   num_devices=NC_CORES)

    xt_d = nc.dram_tensor("xt", [M, S], f16, kind="ExternalInput").ap()
    wq_d = nc.dram_tensor("wq", [M, HG, HD], f16, kind="ExternalInput").ap()
    wk_d = nc.dram_tensor("wk", [M, KVG, HD], f16, kind="ExternalInput").ap()
    wv_d = nc.dram_tensor("wv", [M, KVG, HD], f16, kind="ExternalInput").ap()
    wo_d = nc.dram_tensor("wo", [HD, HG, M], f16, kind="ExternalInput").ap()
    cos_d = nc.dram_tensor("cose", [HD, S], f32, kind="ExternalInput").ap()
    sin_d = nc.dram_tensor("sins", [HD, S], f32, kind="ExternalInput").ap()
    msk_d = nc.dram_tensor("masks", [8, 128, TQ], f16, kind="ExternalInput").ap()
    esk_d = nc.dram_tensor("esink", [128, HG], f32, kind="ExternalInput").ap()
    out_d = nc.dram_tensor("out", [C, TQ // 4, M], f16,
                           kind="ExternalOutput").ap()

    import contextlib
    with tile.TileContext(nc) as tc:
        ctx = contextlib.ExitStack()
        with ctx:
            const = ctx.enter_context(tc.tile_pool(name="const", bufs=1))
            wres = ctx.enter_context(tc.tile_pool(name="wres", bufs=1))
            kvres = ctx.enter_context(tc.tile_pool(name="kvres", bufs=1))
            sxt = ctx.enter_context(tc.tile_pool(name="sxt", bufs=8))
            swkv = ctx.enter_context(tc.tile_pool(name="swkv", bufs=3))
            swo = ctx.enter_context(tc.tile_pool(name="swo", bufs=2))
            sq = ctx.enter_context(tc.tile_pool(name="sq", bufs=2))
            sexp = ctx.enter_context(tc.tile_pool(name="sexp", bufs=16))
            satt = ctx.enter_context(tc.tile_pool(name="satt", bufs=2))
            sden = ctx.enter_context(tc.tile_pool(name="sden", bufs=2))
            srt = ctx.enter_context(tc.tile_pool(name="srt", bufs=2))
            sev = ctx.enter_context(tc.tile_pool(name="sev", bufs=3))
            pps = ctx.enter_context(tc.tile_pool(name="pps", bufs=8, space="PSUM"))
            dram = ctx.enter_context(tc.tile_pool(name="dram", bufs=1, space="DRAM"))

            # ---- constants ----
            cos_sb = const.tile([128, S], f32)
            nc.scalar.dma_start(out=cos_sb, in_=cos_d)
            sin_sb = const.tile([128, S], f32)
            nc.scalar.dma_start(out=sin_sb, in_=sin_d)
            msk_sb = const.tile([128, 8, TQ], f16)
            nc.scalar.dma_start(out=msk_sb, in_=msk_d.rearrange("j p q -> p j q"))
            esk_sb = const.tile([128, HG], f32)
            nc.scalar.dma_start(out=esk_sb, in_=esk_d)
            ones_sb = const.tile([128, 128], f16)
            nc.vector.memset(ones_sb, 1.0)
            bias_sb = const.tile([128, 1], f32)
            nc.vector.memset(bias_sb, -SHIFT)

            # ---- resident weights: Wq (per-mt tiles so matmuls start early) ----
            wq_sb = []
            for mt in range(NMT):
                wqt = wres.tile([128, HG, HD], f16, tag=f"wq{mt}", name=f"wq{mt}")
                nc.scalar.dma_start(out=wqt, in_=wq_d[mt * 128:(mt + 1) * 128, :, :])
                wq_sb.append(wqt)

            # ---- K^T / V caches (full sequence, this core's kv heads) ----
            kT = [kvres.tile([128, S], f16, tag=f"kT{v}", name=f"kT{v}")
                  for v in range(KVG)]
            v_sb = kvres.tile([128, NKT, KVG, HD], f16, tag="v_sb")

            def rope(ps, out_slice, c):
                """out = ps*cos + rot_half(ps)*sin for chunk c (layout [hd, tok])."""
                lo, hi = c * TQ, (c + 1) * TQ
                t1 = srt.tile([128, TQ], f32, tag="rt1")
                nc.vector.tensor_mul(t1, ps, cos_sb[:, lo:hi])
                t2 = srt.tile([128, TQ], f32, tag="rt2")
                nc.vector.tensor_mul(t2[0:64, :], ps[64:128, :], sin_sb[0:64, lo:hi])
                nc.vector.tensor_mul(t2[64:128, :], ps[0:64, :], sin_sb[64:128, lo:hi])
                nc.vector.tensor_add(out_slice, t1, t2)

            rs_outs = []
            for c in range(C):
                lo, hi = c * TQ, (c + 1) * TQ
                kts = _key_tiles(c)

                # ---- pass A: Q projection (Q^T per head) ----
                psq = [pps.tile([128, TQ], f32, tag="ps", name=f"psq{c}_{u}")
                       for u in range(HG)]
                for mt in range(NMT):
                    xa = sxt.tile([128, TQ], f16, tag="xt")
                    nc.sync.dma_start(
                        out=xa, in_=xt_d[mt * 128:(mt + 1) * 128, lo:hi])
                    for u in range(HG):
                        nc.tensor.matmul(psq[u], lhsT=wq_sb[mt][:, u, :], rhs=xa,
                                         start=(mt == 0), stop=(mt == NMT - 1))
                qT = sq.tile([128, HG, TQ], f16, tag="qT")
                for u in range(HG):
                    rope(psq[u], qT[:, u, :], c)

                # ---- pass B: K^T and V projections ----
                psk = [pps.tile([128, TQ], f32, tag="ps", name=f"psk{c}_{v}")
                       for v in range(KVG)]
                psv = [pps.tile([128, KVG * HD], f32, tag="ps", name=f"psv{c}_{t}")
                       for t in range(4)]
                for mt in range(NMT):
                    xb = sxt.tile([128, TQ], f16, tag="xt")
                    nc.sync.dma_start(
                        out=xb, in_=xt_d[mt * 128:(mt + 1) * 128, lo:hi])
                    wk_t = swkv.tile([128, KVG, HD], f16, tag="wk")
                    nc.scalar.dma_start(out=wk_t,
                                      in_=wk_d[mt * 128:(mt + 1) * 128, :, :])
                    wv_t = swkv.tile([128, KVG * HD], f16, tag="wv")
                    nc.scalar.dma_start(
                        out=wv_t,
                        in_=wv_d[mt * 128:(mt + 1) * 128, :, :].rearrange(
                            "p v h -> p (v h)"))
                    for v in range(KVG):
                        nc.tensor.matmul(psk[v], lhsT=wk_t[:, v, :], rhs=xb,
                                         start=(mt == 0), stop=(mt == NMT - 1))
                    for tt in range(4):
                        nc.tensor.matmul(psv[tt],
                                         lhsT=xb[:, tt * 128:(tt + 1) * 128],
                                         rhs=wv_t,
                                         start=(mt == 0), stop=(mt == NMT - 1))
                for v in range(KVG):
                    rope(psk[v], kT[v][:, lo:hi], c)
                for tt in range(4):
                    nc.vector.tensor_copy(
                        v_sb[:, 4 * c + tt, :, :].rearrange("p v h -> p (v h)"),
                        psv[tt])

                # ---- attention (scores transposed: [keys, q]) ----
                attn_sb = satt.tile([128, HG, TQ], f16, tag="attn")
                for u in range(HG):
                    v = u // 4
                    eps = []
                    for kt in kts:
                        pss = pps.tile([128, TQ], f32, tag="ps")
                        nc.tensor.matmul(pss,
                                         lhsT=kT[v][:, kt * 128:(kt + 1) * 128],
                                         rhs=qT[:, u, :], start=True, stop=True)
                        ep = sexp.tile([128, TQ], f16, tag="expp")
                        nc.scalar.activation(ep, pss, Exp, bias=bias_sb,
                                             scale=float(INV_NORM))
                        rel = kt - 4 * c
                        if rel >= 0:
                            nc.vector.tensor_mul(ep, ep, msk_sb[:, rel, :])
                        elif rel < -4:
                            nc.vector.tensor_mul(ep, ep, msk_sb[:, rel + 12, :])
                        eps.append(ep)
                    psd = pps.tile([128, TQ], f32, tag="ps")
                    psa = pps.tile([128, TQ], f32, tag="ps")
                    n = len(kts)
                    for i, (kt, ep) in enumerate(zip(kts, eps)):
                        nc.tensor.matmul(psd, lhsT=ones_sb, rhs=ep,
                                         start=(i == 0), stop=(i == n - 1))
                        nc.tensor.matmul(psa, lhsT=v_sb[:, kt, v, :], rhs=ep,
                                         start=(i == 0), stop=(i == n - 1))
                    den = sden.tile([128, TQ], f32, tag="den")
                    nc.vector.tensor_scalar_add(den, in0=psd,
                                                scalar1=esk_sb[:, u:u + 1])
                    rec = sden.tile([128, TQ], f32, tag="rec")
                    nc.vector.reciprocal(rec, den)
                    nc.vector.tensor_mul(attn_sb[:, u, :], psa, rec)

                # ---- O projection -> fp32 partial rows (per-chunk tensor) ----
                partial = dram.tile([TQ, M], f16, tag=f"part{c}",
                                    name=f"part{c}")
                for ms in range(8):
                    wo_t = swo.tile([128, HG, 512], f16, tag="wo")
                    nc.scalar.dma_start(out=wo_t,
                                      in_=wo_d[:, :, ms * 512:(ms + 1) * 512])
                    for qt in range(4):
                        pso = pps.tile([128, 512], f32, tag="ps")
                        for u in range(HG):
                            nc.tensor.matmul(
                                pso,
                                lhsT=attn_sb[:, u, qt * 128:(qt + 1) * 128],
                                rhs=wo_t[:, u, :],
                                start=(u == 0), stop=(u == HG - 1))
                        ev = sev.tile([128, 512], f16, tag="ev")
                        nc.vector.tensor_copy(ev, pso)
                        nc.gpsimd.dma_start(
                            out=partial[qt * 128:(qt + 1) * 128,
                                        ms * 512:(ms + 1) * 512],
                            in_=ev)

                # ---- ReduceScatter across the batch quad ----
                rs_t = dram.tile([TQ // 4, M], f16, tag=f"rs{c}",
                                 name=f"rs{c}")
                nc.gpsimd.collective_compute(
                    "ReduceScatter", mybir.AluOpType.add,
                    replica_groups=GROUPS,
                    ins=[partial], outs=[rs_t])
                nc.gpsimd.dma_start(out=out_d[c], in_=rs_t)
                rs_outs.append(rs_t)

    nc.compile()
    return nc


def _prep_inputs(hidden_states, Wq, Wk, Wv, Wo, sinks):
    """Build the 8 per-core input maps (numpy only)."""
    half = HD // 2
    inv_freq = 1.0 / (MAX_WAVELENGTH ** (np.arange(half, dtype=np.float32) * 2.0 / HD))
    pos = np.arange(S, dtype=np.float32)
    freq = np.einsum("s,d->ds", pos, inv_freq).astype(np.float32)  # [64, S]
    cos = np.concatenate([np.cos(freq), np.cos(freq)], axis=0).astype(np.float32)
    sinv = np.sin(freq).astype(np.float32)
    sins = np.concatenate([-sinv, sinv], axis=0).astype(np.float32)  # [128, S]

    p = np.arange(128, dtype=np.int64)[:, None]
    q = np.arange(TQ, dtype=np.int64)[None, :]
    masks = np.empty((8, 128, TQ), dtype=np.float16)
    for j in range(4):
        masks[j] = (q >= 128 * j + p).astype(np.float16)       # causal edge
        masks[4 + j] = (q < 128 * j + p).astype(np.float16)    # window edge

    in_maps = []
    for core in range(NC_CORES):
        b, g = core // 4, core % 4
        hs = np.ascontiguousarray(hidden_states[b].T).astype(np.float16)  # [M, S]
        wq = Wq[:, g * HG:(g + 1) * HG, :].astype(np.float16)
        wk = Wk[:, g * KVG:(g + 1) * KVG, :].astype(np.float16)
        wv = Wv[:, g * KVG:(g + 1) * KVG, :].astype(np.float16)
        wo = np.ascontiguousarray(
            Wo[g * HG:(g + 1) * HG].transpose(1, 0, 2)).astype(np.float16)  # [HD, HG, M]
        esink = np.exp(sinks[g * HG:(g + 1) * HG].astype(np.float64) - SHIFT)
        esink = np.broadcast_to(esink.astype(np.float32), (128, HG)).copy()
        in_maps.append({
            "xt": hs, "wq": wq, "wk": wk, "wv": wv, "wo": wo,
            "cose": cos, "sins": sins, "masks": masks, "esink": esink,
        })
    return in_maps


def _get_exec():
    """Build (once) the sharded jitted executor over 8 cores."""
    if "fn" in _built:
        return _built["fn"]
    import jax
    from jax.sharding import Mesh, PartitionSpec
    from jax.experimental.shard_map import shard_map
    from concourse import bass2jax, mybir

    if "nc" not in _built:
        _built["nc"] = _build()
    nc = _built["nc"]
    bass2jax.install_neuronx_cc_hook()

    part_name = nc.partition_id_tensor.name if nc.partition_id_tensor else None
    in_names, out_names, out_avals = [], [], []
    for alloc in nc.m.functions[0].allocations:
        if not isinstance(alloc, mybir.MemoryLocationSet):
            continue
        name = alloc.memorylocations[0].name
        if alloc.kind == "ExternalInput":
            if name != part_name:
                in_names.append(name)
        elif alloc.kind == "ExternalOutput":
            shape = tuple(alloc.tensor_shape)
            out_avals.append(jax.core.ShapedArray(shape, mybir.dt.np(alloc.dtype)))
            out_names.append(name)
    all_in = in_names + out_names
    if part_name is not None:
        all_in = all_in + [part_name]

    def _body(*args):
        operands = list(args)
        if part_name is not None:
            operands.append(bass2jax.partition_id_tensor())
        outs = bass2jax._bass_exec_p.bind(
            *operands,
            out_avals=tuple(out_avals),
            in_names=tuple(all_in),
            out_names=tuple(out_names),
            lowering_input_output_aliases=(),
            sim_require_finite=True,
            sim_require_nnan=True,
            nc=nc,
        )
        return tuple(outs)

    devices = jax.devices()[:NC_CORES]
    mesh = Mesh(np.asarray(devices), ("core",))
    nin = len(in_names) + len(out_names)
    sharded = jax.jit(
        shard_map(_body, mesh=mesh,
                  in_specs=(PartitionSpec("core"),) * nin,
                  out_specs=(PartitionSpec("core"),) * len(out_names),
                  check_rep=False),
        keep_unused=True,
    )
    _built["fn"] = (sharded, in_names, out_names, out_avals, mesh)
    return _built["fn"]


def _concat_inputs(in_maps, in_names, out_avals):
    concat_in = [
        np.concatenate([np.asarray(in_maps[c][n]) for c in range(NC_CORES)], axis=0)
        for n in in_names
    ]
    concat_zeros = [
        np.zeros((NC_CORES * a.shape[0], *a.shape[1:]), a.dtype) for a in out_avals
    ]
    return concat_in, concat_zeros


def _assemble(out_arrs, out_avals):
    per_core = np.asarray(out_arrs[0]).reshape(NC_CORES, *out_avals[0].shape)
    out = np.empty((B, S, M), dtype=np.float32)
    for core in range(NC_CORES):
        b, g = core // 4, core % 4
        o = per_core[core]  # [C, 128, M] fp16
        for c in range(C):
            r0 = c * TQ + g * (TQ // 4)
            out[b, r0:r0 + TQ // 4, :] = o[c].astype(np.float32)
    return out


def kernel(hidden_states, attention_mask, Wq, Wk, Wv, Wo, sinks):
    sharded, in_names, out_names, out_avals, mesh = _get_exec()
    in_maps = _prep_inputs(np.asarray(hidden_states), np.asarray(Wq),
                           np.asarray(Wk), np.asarray(Wv), np.asarray(Wo),
                           np.asarray(sinks))
    concat_in, concat_zeros = _concat_inputs(in_maps, in_names, out_avals)
    out_arrs = sharded(*concat_in, *concat_zeros)
    return _assemble(out_arrs, out_avals)


def time_device(inputs, reps=8):
    """Min wall-clock of repeated executions with device-resident inputs (ns)."""
    import time
    import jax
    from jax.sharding import NamedSharding, PartitionSpec

    sharded, in_names, out_names, out_avals, mesh = _get_exec()
    in_maps = _prep_inputs(np.asarray(inputs["hidden_states"]),
                           np.asarray(inputs["Wq"]), np.asarray(inputs["Wk"]),
                           np.asarray(inputs["Wv"]), np.asarray(inputs["Wo"]),
                           np.asarray(inputs["sinks"]))
    concat_in, concat_zeros = _concat_inputs(in_maps, in_names, out_avals)
    sh = NamedSharding(mesh, PartitionSpec("core"))
    dev_args = [jax.device_put(a, sh) for a in (*concat_in, *concat_zeros)]
    jax.block_until_ready(dev_args)
    out = sharded(*dev_args)           # warm
    jax.block_until_ready(out)
    best = float("inf")
    for _ in range(reps):
        t0 = time.perf_counter()
        out = sharded(*dev_args)
        jax.block_until_ready(out)
        best = min(best, time.perf_counter() - t0)
    return best * 1e9


# revision 31
# speedup vs baseline: 1.1511x; 1.1511x over previous
"""GPT-OSS sliding-window attention (B=2, S=2048, M=4096, 32 q-heads / 8 kv-heads,
window=1024, attention sinks) on 8 trn2 NeuronCores.

Sharding: core = (batch b, head-group g) with b = core//4, g = core%4.
Each core computes 8 q-heads (2 kv-heads) over the full sequence for its batch,
projects through its Wo slice, and the 4 cores of a batch ReduceScatter the
partial [2048, 4096] outputs over the sequence dim -> each core owns disjoint
output rows.  Host-side unshard is a pure gather.

All matmuls run in fp16 (moving/stationary operands) with fp32 PSUM
accumulation; softmax (exp / denominator / reciprocal) in fp32.  Scores are
computed transposed ([keys, q]) so probabilities feed the PV and the
denominator (all-ones lhsT) matmuls directly, with no transposes anywhere.
The causal + sliding-window structure of the mask is exploited: only key-tiles
intersecting the window are computed, and only the 8 boundary-tile patterns
(4 causal-edge + 4 window-edge) are masked, via constant 0/1 fp16 tiles.
"""

import numpy as np

B, S, M = 2, 2048, 4096
NQ, NKV, HD = 32, 8, 128
WINDOW = 1024
MAX_WAVELENGTH = 10000.0
INV_NORM = 1.0 / np.sqrt(HD)
SHIFT = 6.0          # softmax logit shift: exp(s/sqrt(d) - SHIFT), folded into denom+sinks
NC_CORES = 8
HG = 8               # q heads per core
KVG = 2              # kv heads per core
C = 4                # q chunks per sequence
TQ = S // C          # 512 tokens per chunk
NMT = M // 128       # 32 contraction tiles for projections
NKT = S // 128       # 16 key tiles
GROUPS = [[0, 1, 2, 3], [4, 5, 6, 7]]

_built = {}


def _key_tiles(c):
    """Key-tile indices intersecting the causal+sliding window of chunk c."""
    return list(range(max(0, 4 * c - 8), 4 * c + 4))


def _build():
    import concourse.bass as bass
    import concourse.tile as tile
    from concourse import bacc, mybir

    dt = mybir.dt
    f32, f16 = dt.float32, dt.float16
    Exp = mybir.ActivationFunctionType.Exp

    nc = bacc.Bacc("TRN2", target_bir_lowering=False, debug=False,
                   num_devices=NC_CORES)

    xt_d = nc.dram_tensor("xt", [M, S], f16, kind="ExternalInput").ap()
    wq_d = nc.dram_tensor("wq", [M, HG, HD], f16, kind="ExternalInput").ap()
    wkv_d = nc.dram_tensor("wkv", [M, 2, KVG, HD], f16, kind="ExternalInput").ap()
    wo_d = nc.dram_tensor("wo", [HD, HG, M], f16, kind="ExternalInput").ap()
    cos_d = nc.dram_tensor("cose", [HD, S], f16, kind="ExternalInput").ap()
    sin_d = nc.dram_tensor("sins", [HD, S], f16, kind="ExternalInput").ap()
    msk_d = nc.dram_tensor("masks", [8, 128, TQ], f16, kind="ExternalInput").ap()
    esk_d = nc.dram_tensor("esink", [1, HG, 512], f16, kind="ExternalInput").ap()
    out_d = nc.dram_tensor("out", [C, TQ // 4, M], f16,
                           kind="ExternalOutput").ap()

    import contextlib
    with tile.TileContext(nc) as tc:
        ctx = contextlib.ExitStack()
        with ctx:
            const = ctx.enter_context(tc.tile_pool(name="const", bufs=1))
            wres = ctx.enter_context(tc.tile_pool(name="wres", bufs=1))
            kvres = ctx.enter_context(tc.tile_pool(name="kvres", bufs=1))
            sxt = ctx.enter_context(tc.tile_pool(name="sxt", bufs=1))
            swkv = ctx.enter_context(tc.tile_pool(name="swkv", bufs=3))
            swo = ctx.enter_context(tc.tile_pool(name="swo", bufs=2))
            sq = ctx.enter_context(tc.tile_pool(name="sq", bufs=1))
            sexp = ctx.enter_context(tc.tile_pool(name="sexp", bufs=16))
            satt = ctx.enter_context(tc.tile_pool(name="satt", bufs=2))
            sden = ctx.enter_context(tc.tile_pool(name="sden", bufs=2))
            srt = ctx.enter_context(tc.tile_pool(name="srt", bufs=2))
            sev = ctx.enter_context(tc.tile_pool(name="sev", bufs=3))
            pps = ctx.enter_context(tc.tile_pool(name="pps", bufs=8, space="PSUM"))
            dram = ctx.enter_context(tc.tile_pool(name="dram", bufs=1, space="DRAM"))

            # ---- constants (DMAs deferred into chunk-0 pass A) ----
            cos_sb = const.tile([128, S], f16)
            sin_sb = const.tile([128, S], f16)
            msk_sb = const.tile([128, 8, TQ], f16)
            esk_sb = const.tile([1, HG, 512], f16)
            ones_sb = const.tile([128, 128], f16)
            nc.vector.memset(ones_sb, 1.0)
            bias_sb = const.tile([128, 1], f32)
            nc.vector.memset(bias_sb, -SHIFT)

            def load_consts():
                nc.scalar.dma_start(out=cos_sb, in_=cos_d)
                nc.scalar.dma_start(out=sin_sb, in_=sin_d)
                nc.scalar.dma_start(out=msk_sb,
                                    in_=msk_d.rearrange("j p q -> p j q"))
                nc.scalar.dma_start(out=esk_sb, in_=esk_d)

            # ---- resident weights: Wq (DMAs emitted inside chunk-0 pass A) ----
            wq_sb = []
            for mt in range(NMT):
                wqt = wres.tile([128, HG, HD], f16, tag=f"wq{mt}", name=f"wq{mt}")
                wq_sb.append(wqt)

            # ---- K^T / V caches (full sequence, this core's kv heads) ----
            KR = 12  # ring slots (sliding window spans at most 3 chunks)
            kT = [kvres.tile([128, KR * 128], f16, tag=f"kT{v}", name=f"kT{v}")
                  for v in range(KVG)]
            v_sb = kvres.tile([128, KR, KVG, HD], f16, tag="v_sb")

            def rope(ps, out_slice, c):
                """out = ps*cos + rot_half(ps)*sin for chunk c (layout [hd, tok])."""
                lo, hi = c * TQ, (c + 1) * TQ
                t1 = srt.tile([128, TQ], f32, tag="rt1")
                nc.vector.tensor_mul(t1, ps, cos_sb[:, lo:hi])
                t2 = srt.tile([128, TQ], f32, tag="rt2")
                nc.vector.tensor_mul(t2[0:64, :], ps[64:128, :], sin_sb[0:64, lo:hi])
                nc.vector.tensor_mul(t2[64:128, :], ps[0:64, :], sin_sb[64:128, lo:hi])
                nc.vector.tensor_add(out_slice, t1, t2)

            rs_outs = []
            for c in range(C):
                lo, hi = c * TQ, (c + 1) * TQ
                kts = _key_tiles(c)

                # ---- pass A: Q projection (Q^T per head) ----
                xas = []
                for mt in range(NMT):
                    xa = sxt.tile([128, TQ], f16, tag=f"xa{mt}",
                                  name=f"xa{c}_{mt}")
                    xas.append(xa)
                    nc.sync.dma_start(
                        out=xa, in_=xt_d[mt * 128:(mt + 1) * 128, lo:hi])
                    if c == 0:
                        nc.scalar.dma_start(
                            out=wq_sb[mt],
                            in_=wq_d[mt * 128:(mt + 1) * 128, :, :])
                qT = sq.tile([128, HG, TQ], f16, tag="qT")
                for grp in range(2):
                    us = range(4 * grp, 4 * grp + 4)
                    psq = [pps.tile([128, TQ], f32, tag="ps",
                                    name=f"psq{c}_{u}") for u in us]
                    for mt in range(NMT):
                        for i, u in enumerate(us):
                            nc.tensor.matmul(psq[i], lhsT=wq_sb[mt][:, u, :],
                                             rhs=xas[mt],
                                             start=(mt == 0),
                                             stop=(mt == NMT - 1))
                    if c == 0 and grp == 0:
                        load_consts()
                    for i, u in enumerate(us):
                        rope(psq[i], qT[:, u, :], c)

                # ---- pass B: K^T and V projections ----
                psk = [pps.tile([128, TQ], f32, tag="ps", name=f"psk{c}_{v}")
                       for v in range(KVG)]
                psv = [pps.tile([128, KVG * HD], f32, tag="ps", name=f"psv{c}_{t}")
                       for t in range(4)]
                for mt in range(NMT):
                    xb = xas[mt]
                    wkv_t = swkv.tile([128, 2, KVG, HD], f16, tag="wkv")
                    eng = nc.sync if mt % 2 == 0 else nc.scalar
                    eng.dma_start(out=wkv_t,
                                  in_=wkv_d[mt * 128:(mt + 1) * 128])
                    for v in range(KVG):
                        nc.tensor.matmul(psk[v], lhsT=wkv_t[:, 0, v, :], rhs=xb,
                                         start=(mt == 0), stop=(mt == NMT - 1))
                    wv_flat = wkv_t[:, 1, :, :].rearrange("p v h -> p (v h)")
                    for tt in range(4):
                        nc.tensor.matmul(psv[tt],
                                         lhsT=xb[:, tt * 128:(tt + 1) * 128],
                                         rhs=wv_flat,
                                         start=(mt == 0), stop=(mt == NMT - 1))
                for v in range(KVG):
                    sl = (4 * c) % KR
                    if sl + 4 <= KR:
                        rope(psk[v], kT[v][:, sl * 128:(sl + 4) * 128], c)
                    else:
                        rope(psk[v], kT[v][:, sl * 128:], c)  # unreachable for KR%4==0
                for tt in range(4):
                    sl = (4 * c + tt) % KR
                    nc.vector.tensor_copy(
                        v_sb[:, sl, :, :].rearrange("p v h -> p (v h)"),
                        psv[tt])

                # ---- attention (scores transposed: [keys, q]) ----
                # software-pipelined: scores+exp for head u overlap den/PV of u-1
                attn_sb = [satt.tile([128, TQ], f16, tag=f"attn{u}",
                                     name=f"attn{c}_{u}") for u in range(HG)]

                def emit_den_pv_step(state):
                    """Emit one (den, pv) matmul pair of the previous head."""
                    if state is None:
                        return
                    u0, eps0, it = state[0], state[1], state[2]
                    if it >= len(eps0):
                        return
                    v0 = u0 // 4
                    kt0, ep0 = kts[it], eps0[it]
                    n = len(kts)
                    nc.tensor.matmul(state[3], lhsT=ones_sb, rhs=ep0,
                                     start=(it == 0), stop=False)
                    nc.tensor.matmul(state[4], lhsT=v_sb[:, kt0 % KR, v0, :],
                                     rhs=ep0,
                                     start=(it == 0), stop=(it == n - 1))
                    state[2] += 1

                def finish_head(state):
                    if state is None:
                        return
                    while state[2] < len(kts):
                        emit_den_pv_step(state)
                    u0 = state[0]
                    nc.tensor.matmul(state[3], lhsT=ones_sb[0:1, :],
                                     rhs=esk_sb[:, u0, :],
                                     start=False, stop=True)
                    rec = sden.tile([128, TQ], f32, tag="rec")
                    nc.vector.reciprocal(rec, state[3])
                    nc.vector.tensor_mul(attn_sb[u0], state[4], rec)

                pend = None
                for u in range(HG):
                    v = u // 4
                    eps = []
                    for kt in kts:
                        rel = kt - 4 * c
                        midx = rel if rel >= 0 else (rel + 12 if rel < -4 else None)
                        pss = pps.tile([128, TQ], f32, tag="ps")
                        ks = (kt % KR) * 128
                        nc.tensor.matmul(pss,
                                         lhsT=kT[v][:, ks:ks + 128],
                                         rhs=qT[:, u, :], start=True, stop=True)
                        ep = sexp.tile([128, TQ], f16, tag="expp")
                        nc.scalar.activation(ep, pss, Exp, bias=bias_sb,
                                             scale=float(INV_NORM))
                        if midx is not None:
                            nc.vector.tensor_mul(ep, ep, msk_sb[:, midx, :])
                        eps.append(ep)
                        emit_den_pv_step(pend)
                    finish_head(pend)
                    psd = pps.tile([128, TQ], f32, tag="ps", name=f"psd{c}_{u}")
                    psa = pps.tile([128, TQ], f32, tag="ps", name=f"psa{c}_{u}")
                    pend = [u, eps, 0, psd, psa]
                finish_head(pend)

                # ---- O projection -> fp16 partial rows; RS per column-half
                # on the last chunk so the tail collective starts early ----
                nsplit = 2 if c == C - 1 else 1
                mcols = M // nsplit
                parts = [dram.tile([TQ, mcols], f16, tag=f"part{c}_{h}",
                                   name=f"part{c}_{h}") for h in range(nsplit)]
                for ms in range(8):
                    wo_t = swo.tile([128, HG, 512], f16, tag="wo")
                    eng = nc.sync if ms % 2 == 0 else nc.scalar
                    eng.dma_start(out=wo_t,
                                  in_=wo_d[:, :, ms * 512:(ms + 1) * 512])
                    for qt in range(4):
                        pso = pps.tile([128, 512], f32, tag="ps")
                        for u in range(HG):
                            nc.tensor.matmul(
                                pso,
                                lhsT=attn_sb[u][:, qt * 128:(qt + 1) * 128],
                                rhs=wo_t[:, u, :],
                                start=(u == 0), stop=(u == HG - 1))
                        ev = sev.tile([128, 512], f16, tag="ev")
                        nc.vector.tensor_copy(ev, pso)
                        h = (ms * 512) // mcols
                        nc.gpsimd.dma_start(
                            out=parts[h][qt * 128:(qt + 1) * 128,
                                         ms * 512 - h * mcols:
                                         (ms + 1) * 512 - h * mcols],
                            in_=ev)
                    if nsplit > 1 and ms == 3:
                        rs_t = dram.tile([TQ // 4, mcols], f16,
                                         tag=f"rs{c}_0", name=f"rs{c}_0")
                        nc.gpsimd.collective_compute(
                            "ReduceScatter", mybir.AluOpType.add,
                            replica_groups=GROUPS,
                            ins=[parts[0]], outs=[rs_t])
                        nc.gpsimd.dma_start(out=out_d[c][:, 0:mcols], in_=rs_t)
                        rs_outs.append(rs_t)

                # ---- ReduceScatter (remaining columns) across the quad ----
                h = nsplit - 1
                rs_t = dram.tile([TQ // 4, mcols], f16, tag=f"rs{c}_{h}",
                                 name=f"rs{c}_{h}")
                nc.gpsimd.collective_compute(
                    "ReduceScatter", mybir.AluOpType.add,
                    replica_groups=GROUPS,
                    ins=[parts[h]], outs=[rs_t])
                nc.gpsimd.dma_start(out=out_d[c][:, h * mcols:(h + 1) * mcols],
                                    in_=rs_t)
                rs_outs.append(rs_t)

    nc.compile()
    return nc


def _prep_inputs(hidden_states, Wq, Wk, Wv, Wo, sinks):
    """Build the 8 per-core input maps (numpy only)."""
    half = HD // 2
    inv_freq = 1.0 / (MAX_WAVELENGTH ** (np.arange(half, dtype=np.float32) * 2.0 / HD))
    pos = np.arange(S, dtype=np.float32)
    freq = np.einsum("s,d->ds", pos, inv_freq).astype(np.float32)  # [64, S]
    cos = np.concatenate([np.cos(freq), np.cos(freq)], axis=0).astype(np.float16)
    sinv = np.sin(freq).astype(np.float32)
    sins = np.concatenate([-sinv, sinv], axis=0).astype(np.float16)  # [128, S]

    p = np.arange(128, dtype=np.int64)[:, None]
    q = np.arange(TQ, dtype=np.int64)[None, :]
    masks = np.empty((8, 128, TQ), dtype=np.float16)
    for j in range(4):
        masks[j] = (q >= 128 * j + p).astype(np.float16)       # causal edge
        masks[4 + j] = (q < 128 * j + p).astype(np.float16)    # window edge

    in_maps = []
    for core in range(NC_CORES):
        b, g = core // 4, core % 4
        hs = np.ascontiguousarray(hidden_states[b].T).astype(np.float16)  # [M, S]
        wq = Wq[:, g * HG:(g + 1) * HG, :].astype(np.float16)
        wkv = np.stack([Wk[:, g * KVG:(g + 1) * KVG, :],
                        Wv[:, g * KVG:(g + 1) * KVG, :]],
                       axis=1).astype(np.float16)  # [M, 2, KVG, HD]
        wo = np.ascontiguousarray(
            Wo[g * HG:(g + 1) * HG].transpose(1, 0, 2)).astype(np.float16)  # [HD, HG, M]
        esink = np.exp(sinks[g * HG:(g + 1) * HG].astype(np.float64) - SHIFT)
        esink = np.broadcast_to(esink.astype(np.float16)[None, :, None],
                                (1, HG, 512)).copy()
        in_maps.append({
            "xt": hs, "wq": wq, "wkv": wkv, "wo": wo,
            "cose": cos, "sins": sins, "masks": masks, "esink": esink,
        })
    return in_maps


def _get_exec():
    """Build (once) the sharded jitted executor over 8 cores."""
    if "fn" in _built:
        return _built["fn"]
    import jax
    from jax.sharding import Mesh, PartitionSpec
    from jax.experimental.shard_map import shard_map
    from concourse import bass2jax, mybir

    if "nc" not in _built:
        _built["nc"] = _build()
    nc = _built["nc"]
    bass2jax.install_neuronx_cc_hook()

    part_name = nc.partition_id_tensor.name if nc.partition_id_tensor else None
    in_names, out_names, out_avals = [], [], []
    for alloc in nc.m.functions[0].allocations:
        if not isinstance(alloc, mybir.MemoryLocationSet):
            continue
        name = alloc.memorylocations[0].name
        if alloc.kind == "ExternalInput":
            if name != part_name:
                in_names.append(name)
        elif alloc.kind == "ExternalOutput":
            shape = tuple(alloc.tensor_shape)
            out_avals.append(jax.core.ShapedArray(shape, mybir.dt.np(alloc.dtype)))
            out_names.append(name)
    all_in = in_names + out_names
    if part_name is not None:
        all_in = all_in + [part_name]

    def _body(*args):
        operands = list(args)
        if part_name is not None:
            operands.append(bass2jax.partition_id_tensor())
        outs = bass2jax._bass_exec_p.bind(
            *operands,
            out_avals=tuple(out_avals),
            in_names=tuple(all_in),
            out_names=tuple(out_names),
            lowering_input_output_aliases=(),
            sim_require_finite=True,
            sim_require_nnan=True,
            nc=nc,
        )
        return tuple(outs)

    devices = jax.devices()[:NC_CORES]
    mesh = Mesh(np.asarray(devices), ("core",))
    nin = len(in_names) + len(out_names)
    sharded = jax.jit(
        shard_map(_body, mesh=mesh,
                  in_specs=(PartitionSpec("core"),) * nin,
                  out_specs=(PartitionSpec("core"),) * len(out_names),
                  check_rep=False),
        keep_unused=True,
    )
    _built["fn"] = (sharded, in_names, out_names, out_avals, mesh)
    return _built["fn"]


def _concat_inputs(in_maps, in_names, out_avals):
    concat_in = [
        np.concatenate([np.asarray(in_maps[c][n]) for c in range(NC_CORES)], axis=0)
        for n in in_names
    ]
    concat_zeros = [
        np.zeros((NC_CORES * a.shape[0], *a.shape[1:]), a.dtype) for a in out_avals
    ]
    return concat_in, concat_zeros


def _assemble(out_arrs, out_avals):
    per_core = np.asarray(out_arrs[0]).reshape(NC_CORES, *out_avals[0].shape)
    out = np.empty((B, S, M), dtype=np.float32)
    for core in range(NC_CORES):
        b, g = core // 4, core % 4
        o = per_core[core]  # [C, 128, M] fp16
        for c in range(C):
            r0 = c * TQ + g * (TQ // 4)
            out[b, r0:r0 + TQ // 4, :] = o[c].astype(np.float32)
    return out


def kernel(hidden_states, attention_mask, Wq, Wk, Wv, Wo, sinks):
    sharded, in_names, out_names, out_avals, mesh = _get_exec()
    in_maps = _prep_inputs(np.asarray(hidden_states), np.asarray(Wq),
                           np.asarray(Wk), np.asarray(Wv), np.asarray(Wo),
                           np.asarray(sinks))
    concat_in, concat_zeros = _concat_inputs(in_maps, in_names, out_avals)
    out_arrs = sharded(*concat_in, *concat_zeros)
    return _assemble(out_arrs, out_avals)


def time_device(inputs, reps=8):
    """Min wall-clock of repeated executions with device-resident inputs (ns)."""
    import time
    import jax
    from jax.sharding import NamedSharding, PartitionSpec

    sharded, in_names, out_names, out_avals, mesh = _get_exec()
    in_maps = _prep_inputs(np.asarray(inputs["hidden_states"]),
                           np.asarray(inputs["Wq"]), np.asarray(inputs["Wk"]),
                           np.asarray(inputs["Wv"]), np.asarray(inputs["Wo"]),
                           np.asarray(inputs["sinks"]))
    concat_in, concat_zeros = _concat_inputs(in_maps, in_names, out_avals)
    sh = NamedSharding(mesh, PartitionSpec("core"))
    dev_args = [jax.device_put(a, sh) for a in (*concat_in, *concat_zeros)]
    jax.block_until_ready(dev_args)
    out = sharded(*dev_args)           # warm
    jax.block_until_ready(out)
    best = float("inf")
    for _ in range(reps):
        t0 = time.perf_counter()
        out = sharded(*dev_args)
        jax.block_until_ready(out)
        best = min(best, time.perf_counter() - t0)
    return best * 1e9


# revision 33
# speedup vs baseline: 1.2164x; 1.0567x over previous
"""GPT-OSS sliding-window attention (B=2, S=2048, M=4096, 32 q-heads / 8 kv-heads,
window=1024, attention sinks) on 8 trn2 NeuronCores.

Sharding: core = (batch b, head-group g) with b = core//4, g = core%4.
Each core computes 8 q-heads (2 kv-heads) over the full sequence for its batch,
projects through its Wo slice, and the 4 cores of a batch ReduceScatter the
partial [2048, 4096] outputs over the sequence dim -> each core owns disjoint
output rows.  Host-side unshard is a pure gather.

All matmuls run in fp16 (moving/stationary operands) with fp32 PSUM
accumulation; softmax (exp / denominator / reciprocal) in fp32.  Scores are
computed transposed ([keys, q]) so probabilities feed the PV and the
denominator (all-ones lhsT) matmuls directly, with no transposes anywhere.
The causal + sliding-window structure of the mask is exploited: only key-tiles
intersecting the window are computed, and only the 8 boundary-tile patterns
(4 causal-edge + 4 window-edge) are masked, via constant 0/1 fp16 tiles.
"""

import numpy as np

B, S, M = 2, 2048, 4096
NQ, NKV, HD = 32, 8, 128
WINDOW = 1024
MAX_WAVELENGTH = 10000.0
INV_NORM = 1.0 / np.sqrt(HD)
SHIFT = 6.0          # softmax logit shift: exp(s/sqrt(d) - SHIFT), folded into denom+sinks
NC_CORES = 8
HG = 8               # q heads per core
KVG = 2              # kv heads per core
C = 4                # q chunks per sequence
TQ = S // C          # 512 tokens per chunk
NMT = M // 128       # 32 contraction tiles for projections
NKT = S // 128       # 16 key tiles
GROUPS = [[0, 1, 2, 3], [4, 5, 6, 7]]

_built = {}


def _key_tiles(c):
    """Key-tile indices intersecting the causal+sliding window of chunk c."""
    return list(range(max(0, 4 * c - 8), 4 * c + 4))


def _build():
    import concourse.bass as bass
    import concourse.tile as tile
    from concourse import bacc, mybir

    dt = mybir.dt
    f32, f16 = dt.float32, dt.float16
    Exp = mybir.ActivationFunctionType.Exp

    nc = bacc.Bacc("TRN2", target_bir_lowering=False, debug=False,
                   num_devices=NC_CORES)

    xt_d = nc.dram_tensor("xt", [M, S], f16, kind="ExternalInput").ap()
    wq_d = nc.dram_tensor("wq", [M, HG, HD], f16, kind="ExternalInput").ap()
    wkv_d = nc.dram_tensor("wkv", [M, 2, KVG, HD], f16, kind="ExternalInput").ap()
    wo_d = nc.dram_tensor("wo", [HD, HG, M], f16, kind="ExternalInput").ap()
    cos_d = nc.dram_tensor("cose", [HD, S], f16, kind="ExternalInput").ap()
    sin_d = nc.dram_tensor("sins", [HD, S], f16, kind="ExternalInput").ap()
    msk_d = nc.dram_tensor("masks", [8, 128, TQ], f16, kind="ExternalInput").ap()
    esk_d = nc.dram_tensor("esink", [1, HG, 512], f16, kind="ExternalInput").ap()
    out_d = nc.dram_tensor("out", [C, TQ // 4, M], f16,
                           kind="ExternalOutput").ap()

    import contextlib
    with tile.TileContext(nc) as tc:
        ctx = contextlib.ExitStack()
        with ctx:
            const = ctx.enter_context(tc.tile_pool(name="const", bufs=1))
            wres = ctx.enter_context(tc.tile_pool(name="wres", bufs=1))
            kvres = ctx.enter_context(tc.tile_pool(name="kvres", bufs=1))
            sxt = ctx.enter_context(tc.tile_pool(name="sxt", bufs=1))
            swkv = ctx.enter_context(tc.tile_pool(name="swkv", bufs=3))
            swo = ctx.enter_context(tc.tile_pool(name="swo", bufs=2))
            sq = ctx.enter_context(tc.tile_pool(name="sq", bufs=1))
            sexp = ctx.enter_context(tc.tile_pool(name="sexp", bufs=16))
            satt = ctx.enter_context(tc.tile_pool(name="satt", bufs=2))
            sden = ctx.enter_context(tc.tile_pool(name="sden", bufs=2))
            srt = ctx.enter_context(tc.tile_pool(name="srt", bufs=2))
            sev = ctx.enter_context(tc.tile_pool(name="sev", bufs=3))
            pps = ctx.enter_context(tc.tile_pool(name="pps", bufs=8, space="PSUM"))
            dram = ctx.enter_context(tc.tile_pool(name="dram", bufs=1, space="DRAM"))

            # ---- constants (DMAs deferred into chunk-0 pass A) ----
            cos_sb = const.tile([128, S], f16)
            sin_sb = const.tile([128, S], f16)
            msk_sb = const.tile([128, 8, TQ], f16)
            esk_sb = const.tile([1, HG, 512], f16)
            ones_sb = const.tile([128, 128], f16)
            nc.vector.memset(ones_sb, 1.0)
            bias_sb = const.tile([128, 1], f32)
            nc.vector.memset(bias_sb, -SHIFT)

            def load_consts():
                nc.scalar.dma_start(out=cos_sb, in_=cos_d)
                nc.scalar.dma_start(out=sin_sb, in_=sin_d)
                nc.scalar.dma_start(out=msk_sb,
                                    in_=msk_d.rearrange("j p q -> p j q"))
                nc.scalar.dma_start(out=esk_sb, in_=esk_d)

            # ---- resident weights: Wq (DMAs emitted inside chunk-0 pass A) ----
            wq_sb = []
            for mt in range(NMT):
                wqt = wres.tile([128, HG, HD], f16, tag=f"wq{mt}", name=f"wq{mt}")
                wq_sb.append(wqt)

            # ---- K^T / V caches (full sequence, this core's kv heads) ----
            KR = 12  # ring slots (sliding window spans at most 3 chunks)
            kT = [kvres.tile([128, KR * 128], f16, tag=f"kT{v}", name=f"kT{v}")
                  for v in range(KVG)]
            v_sb = kvres.tile([128, KR, KVG, HD], f16, tag="v_sb")

            def rope(ps, out_slice, c):
                """out = ps*cos + rot_half(ps)*sin for chunk c (layout [hd, tok])."""
                lo, hi = c * TQ, (c + 1) * TQ
                t1 = srt.tile([128, TQ], f32, tag="rt1")
                nc.vector.tensor_mul(t1, ps, cos_sb[:, lo:hi])
                t2 = srt.tile([128, TQ], f32, tag="rt2")
                nc.vector.tensor_mul(t2[0:64, :], ps[64:128, :], sin_sb[0:64, lo:hi])
                nc.vector.tensor_mul(t2[64:128, :], ps[0:64, :], sin_sb[64:128, lo:hi])
                nc.vector.tensor_add(out_slice, t1, t2)

            rs_outs = []
            for c in range(C):
                lo, hi = c * TQ, (c + 1) * TQ
                kts = _key_tiles(c)

                # ---- pass A: Q projection (Q^T per head) ----
                xas = []
                for mt in range(NMT):
                    xa = sxt.tile([128, TQ], f16, tag=f"xa{mt}",
                                  name=f"xa{c}_{mt}")
                    xas.append(xa)
                    nc.sync.dma_start(
                        out=xa, in_=xt_d[mt * 128:(mt + 1) * 128, lo:hi])
                    if c == 0:
                        nc.scalar.dma_start(
                            out=wq_sb[mt],
                            in_=wq_d[mt * 128:(mt + 1) * 128, :, :])
                qT = sq.tile([128, HG, TQ], f16, tag="qT")
                for grp in range(2):
                    us = range(4 * grp, 4 * grp + 4)
                    psq = [pps.tile([128, TQ], f32, tag="ps",
                                    name=f"psq{c}_{u}") for u in us]
                    for mt in range(NMT):
                        for i, u in enumerate(us):
                            nc.tensor.matmul(psq[i], lhsT=wq_sb[mt][:, u, :],
                                             rhs=xas[mt],
                                             start=(mt == 0),
                                             stop=(mt == NMT - 1))
                    if c == 0 and grp == 0:
                        load_consts()
                    for i, u in enumerate(us):
                        rope(psq[i], qT[:, u, :], c)

                # ---- pass B: K^T and V projections ----
                psk = [pps.tile([128, TQ], f32, tag="ps", name=f"psk{c}_{v}")
                       for v in range(KVG)]
                psv = [pps.tile([128, KVG * HD], f32, tag="ps", name=f"psv{c}_{t}")
                       for t in range(4)]
                for mt in range(NMT):
                    xb = xas[mt]
                    wkv_t = swkv.tile([128, 2, KVG, HD], f16, tag="wkv")
                    eng = nc.sync if mt % 2 == 0 else nc.scalar
                    eng.dma_start(out=wkv_t,
                                  in_=wkv_d[mt * 128:(mt + 1) * 128])
                    for v in range(KVG):
                        nc.tensor.matmul(psk[v], lhsT=wkv_t[:, 0, v, :], rhs=xb,
                                         start=(mt == 0), stop=(mt == NMT - 1))
                    wv_flat = wkv_t[:, 1, :, :].rearrange("p v h -> p (v h)")
                    for tt in range(4):
                        nc.tensor.matmul(psv[tt],
                                         lhsT=xb[:, tt * 128:(tt + 1) * 128],
                                         rhs=wv_flat,
                                         start=(mt == 0), stop=(mt == NMT - 1))
                for v in range(KVG):
                    sl = (4 * c) % KR
                    if sl + 4 <= KR:
                        rope(psk[v], kT[v][:, sl * 128:(sl + 4) * 128], c)
                    else:
                        rope(psk[v], kT[v][:, sl * 128:], c)  # unreachable for KR%4==0
                for tt in range(4):
                    sl = (4 * c + tt) % KR
                    nc.vector.tensor_copy(
                        v_sb[:, sl, :, :].rearrange("p v h -> p (v h)"),
                        psv[tt])

                # ---- attention (scores transposed: [keys, q]) ----
                # software-pipelined: scores+exp for head u overlap den/PV of u-1
                attn_sb = [satt.tile([128, TQ], f16, tag=f"attn{u}",
                                     name=f"attn{c}_{u}") for u in range(HG)]

                def emit_den_pv_step(state):
                    """Emit one (den, pv) matmul pair of the previous head."""
                    if state is None:
                        return
                    u0, eps0, it = state[0], state[1], state[2]
                    if it >= len(eps0):
                        return
                    v0 = u0 // 4
                    kt0, ep0 = kts[it], eps0[it]
                    n = len(kts)
                    nc.tensor.matmul(state[3], lhsT=ones_sb, rhs=ep0,
                                     start=(it == 0), stop=False)
                    nc.tensor.matmul(state[4], lhsT=v_sb[:, kt0 % KR, v0, :],
                                     rhs=ep0,
                                     start=(it == 0), stop=(it == n - 1))
                    state[2] += 1

                def finish_head(state):
                    if state is None:
                        return
                    while state[2] < len(kts):
                        emit_den_pv_step(state)
                    u0 = state[0]
                    nc.tensor.matmul(state[3], lhsT=ones_sb[0:1, :],
                                     rhs=esk_sb[:, u0, :],
                                     start=False, stop=True)
                    rec = sden.tile([128, TQ], f32, tag="rec")
                    nc.vector.reciprocal(rec, state[3])
                    nc.vector.tensor_mul(attn_sb[u0], state[4], rec)

                pend = None
                for u in range(HG):
                    v = u // 4
                    eps = []
                    for kt in kts:
                        rel = kt - 4 * c
                        midx = rel if rel >= 0 else (rel + 12 if rel < -4 else None)
                        pss = pps.tile([128, TQ], f32, tag="ps")
                        ks = (kt % KR) * 128
                        nc.tensor.matmul(pss,
                                         lhsT=kT[v][:, ks:ks + 128],
                                         rhs=qT[:, u, :], start=True, stop=True)
                        ep = sexp.tile([128, TQ], f16, tag="expp")
                        nc.scalar.activation(ep, pss, Exp, bias=bias_sb,
                                             scale=float(INV_NORM))
                        if midx is not None:
                            nc.vector.tensor_mul(ep, ep, msk_sb[:, midx, :])
                        eps.append(ep)
                        emit_den_pv_step(pend)
                    finish_head(pend)
                    psd = pps.tile([128, TQ], f32, tag="ps", name=f"psd{c}_{u}")
                    psa = pps.tile([128, TQ], f32, tag="ps", name=f"psa{c}_{u}")
                    pend = [u, eps, 0, psd, psa]
                finish_head(pend)

                # ---- O projection -> fp16 partial rows; RS per column-half
                # on the last chunk so the tail collective starts early ----
                nsplit = 2 if c == C - 1 else 1
                mcols = M // nsplit
                parts = [dram.tile([TQ, mcols], f16, tag=f"part{c}_{h}",
                                   name=f"part{c}_{h}") for h in range(nsplit)]
                for ms in range(8):
                    wo_t = swo.tile([128, HG, 512], f16, tag="wo")
                    eng = nc.sync if ms % 2 == 0 else nc.scalar
                    eng.dma_start(out=wo_t,
                                  in_=wo_d[:, :, ms * 512:(ms + 1) * 512])
                    for qt in range(4):
                        pso = pps.tile([128, 512], f32, tag="ps")
                        for u in range(HG):
                            nc.tensor.matmul(
                                pso,
                                lhsT=attn_sb[u][:, qt * 128:(qt + 1) * 128],
                                rhs=wo_t[:, u, :],
                                start=(u == 0), stop=(u == HG - 1))
                        ev = sev.tile([128, 512], f16, tag="ev")
                        nc.vector.tensor_copy(ev, pso)
                        h = (ms * 512) // mcols
                        nc.gpsimd.dma_start(
                            out=parts[h][qt * 128:(qt + 1) * 128,
                                         ms * 512 - h * mcols:
                                         (ms + 1) * 512 - h * mcols],
                            in_=ev)
                    if nsplit > 1 and ms == 3:
                        rs_t = dram.tile([TQ // 4, mcols], f16,
                                         tag=f"rs{c}_0", name=f"rs{c}_0")
                        nc.gpsimd.collective_compute(
                            "ReduceScatter", mybir.AluOpType.add,
                            replica_groups=GROUPS,
                            ins=[parts[0]], outs=[rs_t])
                        nc.gpsimd.dma_start(out=out_d[c][:, 0:mcols], in_=rs_t)
                        rs_outs.append(rs_t)

                # ---- ReduceScatter (remaining columns) across the quad ----
                h = nsplit - 1
                rs_t = dram.tile([TQ // 4, mcols], f16, tag=f"rs{c}_{h}",
                                 name=f"rs{c}_{h}")
                nc.gpsimd.collective_compute(
                    "ReduceScatter", mybir.AluOpType.add,
                    replica_groups=GROUPS,
                    ins=[parts[h]], outs=[rs_t])
                nc.gpsimd.dma_start(out=out_d[c][:, h * mcols:(h + 1) * mcols],
                                    in_=rs_t)
                rs_outs.append(rs_t)

    nc.compile()
    return nc


def _prep_inputs(hidden_states, Wq, Wk, Wv, Wo, sinks):
    """Build the 8 per-core input maps (numpy only)."""
    half = HD // 2
    inv_freq = 1.0 / (MAX_WAVELENGTH ** (np.arange(half, dtype=np.float32) * 2.0 / HD))
    pos = np.arange(S, dtype=np.float32)
    freq = np.einsum("s,d->ds", pos, inv_freq).astype(np.float32)  # [64, S]
    cos = np.concatenate([np.cos(freq), np.cos(freq)], axis=0).astype(np.float16)
    sinv = np.sin(freq).astype(np.float32)
    sins = np.concatenate([-sinv, sinv], axis=0).astype(np.float16)  # [128, S]

    p = np.arange(128, dtype=np.int64)[:, None]
    q = np.arange(TQ, dtype=np.int64)[None, :]
    masks = np.empty((8, 128, TQ), dtype=np.float16)
    for j in range(4):
        masks[j] = (q >= 128 * j + p).astype(np.float16)       # causal edge
        masks[4 + j] = (q < 128 * j + p).astype(np.float16)    # window edge

    in_maps = []
    for core in range(NC_CORES):
        b, g = core // 4, core % 4
        hs = np.ascontiguousarray(hidden_states[b].T).astype(np.float16)  # [M, S]
        wq = Wq[:, g * HG:(g + 1) * HG, :].astype(np.float16)
        wkv = np.stack([Wk[:, g * KVG:(g + 1) * KVG, :],
                        Wv[:, g * KVG:(g + 1) * KVG, :]],
                       axis=1).astype(np.float16)  # [M, 2, KVG, HD]
        wo = np.ascontiguousarray(
            Wo[g * HG:(g + 1) * HG].transpose(1, 0, 2)).astype(np.float16)  # [HD, HG, M]
        esink = np.exp(sinks[g * HG:(g + 1) * HG].astype(np.float64) - SHIFT)
        esink = np.broadcast_to(esink.astype(np.float16)[None, :, None],
                                (1, HG, 512)).copy()
        in_maps.append({
            "xt": hs, "wq": wq, "wkv": wkv, "wo": wo,
            "cose": cos, "sins": sins, "masks": masks, "esink": esink,
        })
    return in_maps


def _get_exec():
    """Build (once) the sharded jitted executor over 8 cores."""
    if "fn" in _built:
        return _built["fn"]
    import jax
    from jax.sharding import Mesh, PartitionSpec
    from jax.experimental.shard_map import shard_map
    from concourse import bass2jax, mybir

    if "nc" not in _built:
        _built["nc"] = _build()
    nc = _built["nc"]
    bass2jax.install_neuronx_cc_hook()

    part_name = nc.partition_id_tensor.name if nc.partition_id_tensor else None
    in_names, out_names, out_avals = [], [], []
    for alloc in nc.m.functions[0].allocations:
        if not isinstance(alloc, mybir.MemoryLocationSet):
            continue
        name = alloc.memorylocations[0].name
        if alloc.kind == "ExternalInput":
            if name != part_name:
                in_names.append(name)
        elif alloc.kind == "ExternalOutput":
            shape = tuple(alloc.tensor_shape)
            out_avals.append(jax.core.ShapedArray(shape, mybir.dt.np(alloc.dtype)))
            out_names.append(name)
    all_in = in_names + out_names
    if part_name is not None:
        all_in = all_in + [part_name]

    def _body(*args):
        operands = list(args)
        if part_name is not None:
            operands.append(bass2jax.partition_id_tensor())
        outs = bass2jax._bass_exec_p.bind(
            *operands,
            out_avals=tuple(out_avals),
            in_names=tuple(all_in),
            out_names=tuple(out_names),
            lowering_input_output_aliases=(),
            sim_require_finite=True,
            sim_require_nnan=True,
            nc=nc,
        )
        return tuple(outs)

    devices = jax.devices()[:NC_CORES]
    mesh = Mesh(np.asarray(devices), ("core",))
    nin = len(in_names) + len(out_names)
    sharded = jax.jit(
        shard_map(_body, mesh=mesh,
                  in_specs=(PartitionSpec("core"),) * nin,
                  out_specs=(PartitionSpec("core"),) * len(out_names),
                  check_rep=False),
        keep_unused=True,
    )
    _built["fn"] = (sharded, in_names, out_names, out_avals, mesh)
    return _built["fn"]


def _concat_inputs(in_maps, in_names, out_avals):
    concat_in = [
        np.concatenate([np.asarray(in_maps[c][n]) for c in range(NC_CORES)], axis=0)
        for n in in_names
    ]
    concat_zeros = [
        np.zeros((NC_CORES * a.shape[0], *a.shape[1:]), a.dtype) for a in out_avals
    ]
    return concat_in, concat_zeros


def _assemble(out_arrs, out_avals):
    per_core = np.asarray(out_arrs[0]).reshape(NC_CORES, *out_avals[0].shape)
    out = np.empty((B, S, M), dtype=np.float32)
    for core in range(NC_CORES):
        b, g = core // 4, core % 4
        o = per_core[core]  # [C, 128, M] fp16
        for c in range(C):
            r0 = c * TQ + g * (TQ // 4)
            out[b, r0:r0 + TQ // 4, :] = o[c].astype(np.float32)
    return out


def kernel(hidden_states, attention_mask, Wq, Wk, Wv, Wo, sinks):
    sharded, in_names, out_names, out_avals, mesh = _get_exec()
    in_maps = _prep_inputs(np.asarray(hidden_states), np.asarray(Wq),
                           np.asarray(Wk), np.asarray(Wv), np.asarray(Wo),
                           np.asarray(sinks))
    concat_in, concat_zeros = _concat_inputs(in_maps, in_names, out_avals)
    out_arrs = sharded(*concat_in, *concat_zeros)
    return _assemble(out_arrs, out_avals)


def time_device(inputs, reps=8):
    """Min wall-clock of repeated executions with device-resident inputs (ns)."""
    import time
    import jax
    from jax.sharding import NamedSharding, PartitionSpec

    sharded, in_names, out_names, out_avals, mesh = _get_exec()
    in_maps = _prep_inputs(np.asarray(inputs["hidden_states"]),
                           np.asarray(inputs["Wq"]), np.asarray(inputs["Wk"]),
                           np.asarray(inputs["Wv"]), np.asarray(inputs["Wo"]),
                           np.asarray(inputs["sinks"]))
    concat_in, concat_zeros = _concat_inputs(in_maps, in_names, out_avals)
    sh = NamedSharding(mesh, PartitionSpec("core"))
    dev_args = [jax.device_put(a, sh) for a in (*concat_in, *concat_zeros)]
    jax.block_until_ready(dev_args)
    out = sharded(*dev_args)           # warm
    jax.block_until_ready(out)
    best = float("inf")
    for _ in range(reps):
        t0 = time.perf_counter()
        out = sharded(*dev_args)
        jax.block_until_ready(out)
        best = min(best, time.perf_counter() - t0)
    return best * 1e9
